# revision 10
# baseline (speedup 1.0000x reference)
"""CompressionGPT Trainium2 kernel.

Model: B=4, T=1024, E=512, H=8 (HD=64), L=4, V=32000.
Sharding across 8 NeuronCores (no collectives):
  core c -> sample c % 4 (transformer replicated across the core pair),
  lm_head vocab half c // 4 (16000 vocab columns per core).
Host does: embedding gather + positional select, attention-mask build,
weight transposition/casting to bf16, NaN fill of encoder rows, output
assembly.

Device layouts (per core):
  t (token)    : t  = to * 128 + p      (to in [0, T/128))
  e (embed)    : e  = eo * 128 + p      (eo in [0, 4))
  f (qk feat)  : f  = fo * 128 + p      (fo in [0, 8); q: f < 512, k: f >= 512)
  f2 (ffn hid) : f2 = go * 128 + p      (go in [0, 16))

Attention per head: scores are computed transposed, sT[k, q] = (kT_h).T@qT_h
with q pre-scaled by 1/sqrt(HD) (folded into w_q on host).  exp() is applied
with NO max subtraction (scores are O(1) for this model family), masked
entries are zeroed by a multiplicative 0/1 bf16 mask, and the softmax
denominator comes from an extra ones-column appended to V, so
(A@V) psum column 64 = sum_k p[k, q]; normalization is folded into the psum
eviction.
"""

import os
import numpy as np
import ml_dtypes

BF16 = ml_dtypes.bfloat16

B, T, E, H, L, V = 4, 1024, 512, 8, 4, 32000
HD = E // H  # 64
P = 128
NCORES = 8
VSHARD = V // 2  # vocab half per core

_CACHE = {}


def _build_nc(cfg):
    """Build the Bass module. cfg: dict(T=, L=, VS=, causal=True)."""
    import concourse.bass as bass
    import concourse.mybir as mybir
    import concourse.tile as tile
    import concourse.bacc as bacc
    from concourse.masks import make_identity
    from contextlib import ExitStack

    f32 = mybir.dt.float32
    bf16 = mybir.dt.bfloat16
    AX = mybir.AxisListType.X
    OP = mybir.AluOpType
    AF = mybir.ActivationFunctionType

    Tn = cfg["T"]
    Ln = cfg["L"]
    VS = cfg["VS"]
    act_gelu = getattr(AF, cfg.get("act", "Gelu"))
    nbias = cfg["nonzero_bias"]
    naff = cfg["nonzero_affine"]
    TO = Tn // P
    EO = E // P
    G = 4 * E // P   # 16 ffn hidden tiles
    NT = min(512, Tn)          # moving-dim chunk for t
    NTC = Tn // NT
    NTF = min(256, Tn)         # ffn t-chunk (smaller: h2T SBUF footprint)
    NTFC = Tn // NTF
    VC = 500                   # lm_head vocab chunk
    assert VS % VC == 0

    nc = bacc.Bacc("TRN2", target_bir_lowering=False, debug=False,
                   num_devices=NCORES)

    x0_d = nc.dram_tensor("x0", [P, TO, E], f32, kind="ExternalInput")
    mask_d = nc.dram_tensor("maskT", [P, TO, Tn], bf16, kind="ExternalInput")
    wqk_d = nc.dram_tensor("wqkT", [Ln, P, EO, 2 * E], bf16, kind="ExternalInput")
    wv_d = nc.dram_tensor("wvT", [Ln, P, EO, E], bf16, kind="ExternalInput")
    wo_d = nc.dram_tensor("woutT", [Ln, P, EO, E], bf16, kind="ExternalInput")
    wf_d = nc.dram_tensor("wfcT", [Ln, P, EO, 4 * E], bf16, kind="ExternalInput")
    wp_d = nc.dram_tensor("wprojT", [Ln, P, G, E], bf16, kind="ExternalInput")
    wh_d = nc.dram_tensor("wheadT", [P, EO, VS], bf16, kind="ExternalInput")
    out_d = nc.dram_tensor("logits", [P, TO, VS], f32, kind="ExternalOutput")
    if nbias:
        # b_qk: [L,P,8] feature-major; b_v/b_out/b_proj replicated [L,P,E];
        # b_fc feature-major [L,P,16]
        bqk_d = nc.dram_tensor("bqk", [Ln, P, 2 * E // P], f32, kind="ExternalInput")
        bv_d = nc.dram_tensor("bvr", [Ln, P, E], f32, kind="ExternalInput")
        bo_d = nc.dram_tensor("bor", [Ln, P, E], f32, kind="ExternalInput")
        bp_d = nc.dram_tensor("bpr", [Ln, P, E], f32, kind="ExternalInput")
        bf_d = nc.dram_tensor("bfc", [Ln, P, G], f32, kind="ExternalInput")
    if naff:
        # replicated LN affine: [nLN, P, E] with order (ln1 x L, ln2 x L, lnf)
        g_d = nc.dram_tensor("lngr", [2 * Ln + 1, P, E], f32, kind="ExternalInput")
        b_d = nc.dram_tensor("lnbr", [2 * Ln + 1, P, E], f32, kind="ExternalInput")

    with tile.TileContext(nc) as tc, ExitStack() as ctx:
        const = ctx.enter_context(tc.tile_pool(name="const", bufs=1))
        xp = ctx.enter_context(tc.tile_pool(name="xp", bufs=1))
        mkp = ctx.enter_context(tc.tile_pool(name="mkp", bufs=1))
        hTp = ctx.enter_context(tc.tile_pool(name="hTp", bufs=2))
        qkp = ctx.enter_context(tc.tile_pool(name="qkp", bufs=1))
        vp = ctx.enter_context(tc.tile_pool(name="vp", bufs=1))
        ptp = ctx.enter_context(tc.tile_pool(name="ptp", bufs=2))
        op_ = ctx.enter_context(tc.tile_pool(name="op", bufs=1))
        otp = ctx.enter_context(tc.tile_pool(name="otp", bufs=2))
        h2p = ctx.enter_context(tc.tile_pool(name="h2p", bufs=2))
        wpool = ctx.enter_context(tc.tile_pool(name="wpool", bufs=1))
        whp = ctx.enter_context(tc.tile_pool(name="whp", bufs=2))
        tmp = ctx.enter_context(tc.tile_pool(name="tmp", bufs=3))
        stat = ctx.enter_context(tc.tile_pool(name="stat", bufs=4))
        lhp = ctx.enter_context(tc.tile_pool(name="lhp", bufs=3))
        psum = ctx.enter_context(tc.tile_pool(name="psum", bufs=6, space="PSUM"))
        psT = ctx.enter_context(tc.tile_pool(name="psT", bufs=2, space="PSUM"))

        ident = const.tile([P, P], bf16)
        make_identity(nc, ident)

        x = xp.tile([P, TO, E], f32)
        nc.sync.dma_start(x[:], x0_d.ap())
        maskT = mkp.tile([P, TO, Tn], bf16)
        nc.sync.dma_start(maskT[:], mask_d.ap())
        if nbias:
            bqk_s = const.tile([P, Ln, 2 * E // P], f32, name="bqk_s")
            nc.sync.dma_start(bqk_s[:], bqk_d.ap().rearrange("l p c -> p l c"))
            # replicated biases are loaded per layer below
            bfc_s = const.tile([P, Ln, G], f32, name="bfc_s")
            nc.sync.dma_start(bfc_s[:], bf_d.ap().rearrange("l p c -> p l c"))

        def layernorm_T(x3, hT, ln_idx):
            """x3: [P, TO, E] f32 (token-major) -> hT [P, EO, Tn] bf16
            (feature-major, transposed), LN over E."""
            if naff:
                g_t = lhp.tile([P, E], f32, tag="g_t")
                b_t = lhp.tile([P, E], f32, tag="b_t")
                nc.sync.dma_start(g_t[:], g_d.ap()[ln_idx])
                nc.sync.dma_start(b_t[:], b_d.ap()[ln_idx])
            for to in range(TO):
                xt = x3[:, to, :]
                ssum = stat.tile([P, 1], f32, tag="ssum")
                nc.vector.tensor_reduce(ssum[:], xt, axis=AX, op=OP.add)
                sq = tmp.tile([P, E], f32, tag="sq")
                sqsum = stat.tile([P, 1], f32, tag="sqsum")
                nc.scalar.activation(sq[:], xt, AF.Square, accum_out=sqsum[:])
                negmean = stat.tile([P, 1], f32, tag="negmean")
                nc.vector.tensor_scalar_mul(negmean[:], ssum[:], -1.0 / E)
                nm2 = stat.tile([P, 1], f32, tag="nm2")
                nc.vector.tensor_tensor(nm2[:], negmean[:], negmean[:], OP.mult)
                var = stat.tile([P, 1], f32, tag="var")
                # var = sqsum/E - mean^2 + eps
                nc.vector.tensor_scalar(var[:], sqsum[:], 1.0 / E, 1e-5,
                                        OP.mult, OP.add)
                nc.vector.tensor_tensor(var[:], var[:], nm2[:], OP.subtract)
                std = stat.tile([P, 1], f32, tag="std")
                nc.scalar.sqrt(std[:], var[:])
                rstd = stat.tile([P, 1], f32, tag="rstd")
                nc.vector.reciprocal(rstd[:], std[:])
                h = tmp.tile([P, E], bf16, tag="h")
                if naff:
                    hf = tmp.tile([P, E], f32, tag="hf")
                    nc.vector.scalar_tensor_tensor(
                        hf[:], xt, negmean[:], rstd[:].to_broadcast((P, E)),
                        OP.add, OP.mult)
                    nc.vector.tensor_tensor(hf[:], hf[:], g_t[:], OP.mult)
                    nc.vector.tensor_tensor(h[:], hf[:], b_t[:], OP.add)
                else:
                    nc.vector.scalar_tensor_tensor(
                        h[:], xt, negmean[:], rstd[:].to_broadcast((P, E)),
                        OP.add, OP.mult)
                for eo in range(EO):
                    pt = psT.tile([P, P], bf16, tag="pt")
                    nc.tensor.transpose(pt[:], h[:, eo * P:(eo + 1) * P], ident[:])
                    nc.scalar.copy(hT[:, eo, to * P:(to + 1) * P], pt[:])

        for li in range(Ln):
            wqk = wpool.tile([P, EO, 2 * E], bf16, tag="wqk")
            nc.sync.dma_start(wqk[:], wqk_d.ap()[li])
            wv = wpool.tile([P, EO, E], bf16, tag="wv")
            nc.sync.dma_start(wv[:], wv_d.ap()[li])
            wo = wpool.tile([P, EO, E], bf16, tag="wo")
            nc.sync.dma_start(wo[:], wo_d.ap()[li])
            wf = wpool.tile([P, EO, 4 * E], bf16, tag="wf")
            nc.sync.dma_start(wf[:], wf_d.ap()[li])
            wp = wpool.tile([P, G, E], bf16, tag="wp")
            nc.sync.dma_start(wp[:], wp_d.ap()[li])
            if nbias:
                bv_t = lhp.tile([P, E], f32, tag="bv_t")
                nc.sync.dma_start(bv_t[:], bv_d.ap()[li])
                bo_t = lhp.tile([P, E], f32, tag="bo_t")
                nc.sync.dma_start(bo_t[:], bo_d.ap()[li])
                bp_t = lhp.tile([P, E], f32, tag="bp_t")
                nc.sync.dma_start(bp_t[:], bp_d.ap()[li])

            # ---- ln1 -> hT
            hT = hTp.tile([P, EO, Tn], bf16, tag="hT")
            layernorm_T(x[:], hT, li)

            # ---- qkT[f, t] = (w_qk/w_k).T-major matmul
            qkT = qkp.tile([P, 2 * EO, Tn], bf16)
            for fo in range(2 * EO):
                for tc_ in range(NTC):
                    ps = psum.tile([P, 512], f32, tag="ps")
                    pss = ps[:, :NT]
                    tsl = slice(tc_ * NT, (tc_ + 1) * NT)
                    for eo in range(EO):
                        nc.tensor.matmul(pss, wqk[:, eo, fo * P:(fo + 1) * P],
                                         hT[:, eo, tsl],
                                         start=(eo == 0), stop=(eo == EO - 1))
                    if nbias:
                        nc.scalar.activation(qkT[:, fo, tsl], pss, AF.Identity,
                                             bias=bqk_s[:, li, fo:fo + 1])
                    else:
                        nc.scalar.copy(qkT[:, fo, tsl], pss)

            # ---- v token-major with ones column per head: [P, TO, H*65]
            vaug = vp.tile([P, TO, H * 65], bf16)
            ones_view = vaug[:].rearrange("p t (h c) -> p t h c", c=65)[:, :, :, 64:65]
            nc.vector.memset(ones_view, 1.0)
            for to in range(TO):
                ps = psum.tile([P, 512], f32, tag="ps")
                for eo in range(EO):
                    nc.tensor.matmul(ps[:], hT[:, eo, to * P:(to + 1) * P],
                                     wv[:, eo, :],
                                     start=(eo == 0), stop=(eo == EO - 1))
                dstv = vaug[:, to, :].rearrange("p (h c) -> p h c", c=65)[:, :, :64]
                srcv = ps[:].rearrange("p (h c) -> p h c", c=64)
                if nbias:
                    nc.vector.tensor_tensor(
                        dstv, srcv,
                        bv_t[:].rearrange("p (h c) -> p h c", c=64), OP.add)
                else:
                    nc.scalar.copy(dstv, srcv)

            # ---- attention per (head, q-chunk)
            osb = op_.tile([P, TO, E], bf16)
            for hh in range(H):
                qoff = (hh * HD) % P
                qfo = (hh * HD) // P
                kfo = EO + qfo
                qT_h = qkT[qoff:qoff + HD, qfo, :]
                kT_h = qkT[qoff:qoff + HD, kfo, :]
                for tc_ in range(NTC):
                    pT = ptp.tile([P, TO, NT], bf16, tag="pT")
                    komax = min(TO, ((tc_ + 1) * NT) // P)
                    tsl = slice(tc_ * NT, (tc_ + 1) * NT)
                    for ko in range(komax):
                        ps = psum.tile([P, 512], f32, tag="ps")
                        pss = ps[:, :NT]
                        nc.tensor.matmul(pss, kT_h[:, ko * P:(ko + 1) * P],
                                         qT_h[:, tsl], start=True, stop=True)
                        nc.scalar.activation(pT[:, ko, :], pss, AF.Exp)
                        nc.vector.tensor_tensor(pT[:, ko, :], pT[:, ko, :],
                                                maskT[:, ko, tsl], OP.mult)
                    for qt in range(tc_ * NT // P, (tc_ + 1) * NT // P):
                        lsl = slice(qt * P - tc_ * NT, (qt + 1) * P - tc_ * NT)
                        ps = psum.tile([P, 512], f32, tag="ps")
                        pso = ps[:, :65]
                        kmax = qt + 1
                        for ko in range(kmax):
                            nc.tensor.matmul(pso, pT[:, ko, lsl],
                                             vaug[:, ko, hh * 65:hh * 65 + 65],
                                             start=(ko == 0),
                                             stop=(ko == kmax - 1))
                        r = stat.tile([P, 1], f32, tag="r")
                        nc.vector.reciprocal(r[:], ps[:, 64:65])
                        nc.vector.tensor_scalar_mul(
                            osb[:, qt, hh * HD:(hh + 1) * HD], ps[:, :HD], r[:])

            # ---- per t-tile: transpose o -> oT, w_out + residual
            for to in range(TO):
                oT = otp.tile([P, EO, P], bf16, tag="oT")
                for fo in range(EO):
                    pt = psT.tile([P, P], bf16, tag="pt")
                    nc.tensor.transpose(pt[:], osb[:, to, fo * P:(fo + 1) * P],
                                        ident[:])
                    nc.scalar.copy(oT[:, fo, :], pt[:])
                ps = psum.tile([P, 512], f32, tag="ps")
                for fo in range(EO):
                    nc.tensor.matmul(ps[:], oT[:, fo, :],
                                     wo[:, fo, :],
                                     start=(fo == 0), stop=(fo == EO - 1))
                if nbias:
                    nc.vector.tensor_tensor(ps[:], ps[:], bo_t[:], OP.add)
                nc.vector.tensor_tensor(x[:, to, :], x[:, to, :], ps[:], OP.add)

            # ---- ln2 -> hT2
            hT2 = hTp.tile([P, EO, Tn], bf16, tag="hT")
            layernorm_T(x[:], hT2, Ln + li)

            # ---- FFN
            for tc_ in range(NTFC):
                tsl = slice(tc_ * NTF, (tc_ + 1) * NTF)
                h2T = h2p.tile([P, G, NTF], bf16, tag="h2T")
                for go in range(G):
                    ps = psum.tile([P, 512], f32, tag="ps")
                    pss = ps[:, :NTF]
                    for eo in range(EO):
                        nc.tensor.matmul(pss, wf[:, eo, go * P:(go + 1) * P],
                                         hT2[:, eo, tsl],
                                         start=(eo == 0), stop=(eo == EO - 1))
                    if nbias:
                        nc.scalar.activation(h2T[:, go, :], pss, act_gelu,
                                             bias=bfc_s[:, li, go:go + 1])
                    else:
                        nc.scalar.activation(h2T[:, go, :], pss, act_gelu)
                for tt in range(NTF // P):
                    to = tc_ * (NTF // P) + tt
                    ps = psum.tile([P, 512], f32, tag="ps")
                    for go in range(G):
                        nc.tensor.matmul(ps[:], h2T[:, go, tt * P:(tt + 1) * P],
                                         wp[:, go, :],
                                         start=(go == 0), stop=(go == G - 1))
                    if nbias:
                        nc.vector.tensor_tensor(ps[:], ps[:], bp_t[:], OP.add)
                    nc.vector.tensor_tensor(x[:, to, :], x[:, to, :], ps[:],
                                            OP.add)

        # ---- final LN -> xfT, lm_head
        xfT = hTp.tile([P, EO, Tn], bf16, tag="hT")
        layernorm_T(x[:], xfT, 2 * Ln)
        for vc in range(VS // VC):
            wh = whp.tile([P, EO, VC], bf16, tag="wh")
            nc.sync.dma_start(wh[:], wh_d.ap()[:, :, vc * VC:(vc + 1) * VC])
            for to in range(TO):
                ps = psum.tile([P, 512], f32, tag="ps")
                pss = ps[:, :VC]
                for eo in range(EO):
                    nc.tensor.matmul(pss, xfT[:, eo, to * P:(to + 1) * P],
                                     wh[:, eo, :],
                                     start=(eo == 0), stop=(eo == EO - 1))
                lo = lhp.tile([P, VC], f32, tag="lo")
                nc.scalar.copy(lo[:], pss)
                nc.sync.dma_start(out_d.ap()[:, to, vc * VC:(vc + 1) * VC], lo[:])

    nc.compile()
    return nc


def _rearr_pt(a):
    """[T_, X] -> [P, T_/P, X] with t = to*128 + p."""
    t, xdim = a.shape
    return np.ascontiguousarray(a.reshape(t // P, P, xdim).transpose(1, 0, 2))


def _host_prep(wte, wpe, wpe_enc, ln1_g, ln1_b, w_in, b_in, w_out, b_out,
               ln2_g, ln2_b, w_fc, b_fc, w_proj, b_proj, lnf_g, lnf_b, w_head,
               tok, idxs_dec):
    """Build per-core input maps + metadata. Returns (in_maps, cfg_flags, d)."""
    tok = np.asarray(tok).astype(np.int64)
    d = np.asarray(idxs_dec).astype(np.int64)

    pos = np.arange(T)
    batch_mask = pos[None, :] < d[:, None]          # (B, T)
    # embeddings
    pe = np.where(batch_mask[:, :, None], wpe_enc[pos][None], wpe[pos][None])
    x0 = (wte[tok] + pe).astype(np.float32)          # (B, T, E)

    # attention mask (B, T, T), bool, mask[b, r, c]
    r = pos[:, None]
    c = pos[None, :]
    dd = d[:, None, None]
    mask = (c <= r) & ~((r >= dd) & (c < dd))
    mask = mask | ((r >= dd) & (c == np.maximum(dd - 1, 0)))
    # transposed multiplicative mask, [B, T(k), T(q)] -> [B, P, TO, T]
    maskT = mask.transpose(0, 2, 1).astype(BF16)

    scale = np.float32(1.0 / np.sqrt(HD))
    wqkT = np.empty((L, P, E // P, 2 * E), BF16)
    wvT = np.empty((L, P, E // P, E), BF16)
    woutT = np.empty((L, P, E // P, E), BF16)
    wfcT = np.empty((L, P, E // P, 4 * E), BF16)
    wprojT = np.empty((L, P, 4 * E // P, E), BF16)
    for i in range(L):
        wq = w_in[i, :E] * scale                     # (512, 512) [f, e]
        wk = w_in[i, E:2 * E]
        wqk = np.concatenate([wq, wk], 0).T          # (e, 2E)
        wqkT[i] = _rearr_pt(wqk.astype(BF16))
        wvT[i] = _rearr_pt(w_in[i, 2 * E:].T.astype(BF16))       # (e, 512)
        woutT[i] = _rearr_pt(w_out[i].T.astype(BF16))            # (f, e)
        wfcT[i] = _rearr_pt(w_fc[i].T.astype(BF16))              # (e, 2048)
        wprojT[i] = _rearr_pt(w_proj[i].T.astype(BF16))          # (f2, e)

    whT = w_head.T.astype(BF16)                       # (e, V)
    whT_halves = [_rearr_pt(np.ascontiguousarray(whT[:, :VSHARD])),
                  _rearr_pt(np.ascontiguousarray(whT[:, VSHARD:]))]

    nonzero_bias = any(np.any(a) for a in (b_in, b_out, b_fc, b_proj))
    nonzero_affine = not (
        np.all(ln1_g == 1) and np.all(ln2_g == 1) and np.all(lnf_g == 1)
        and not np.any(ln1_b) and not np.any(ln2_b) and not np.any(lnf_b))

    base = {
        "wqkT": wqkT, "wvT": wvT, "woutT": woutT, "wfcT": wfcT,
        "wprojT": wprojT,
    }
    if nonzero_bias:
        bqk = np.concatenate([b_in[:, :E] * scale, b_in[:, E:2 * E]], 1)
        base["bqk"] = np.ascontiguousarray(
            bqk.reshape(L, 2 * E // P, P).transpose(0, 2, 1)).astype(np.float32)
        base["bvr"] = np.broadcast_to(
            b_in[:, None, 2 * E:], (L, P, E)).astype(np.float32).copy()
        base["bor"] = np.broadcast_to(
            b_out[:, None, :], (L, P, E)).astype(np.float32).copy()
        base["bpr"] = np.broadcast_to(
            b_proj[:, None, :], (L, P, E)).astype(np.float32).copy()
        base["bfc"] = np.ascontiguousarray(
            b_fc.reshape(L, 4 * E // P, P).transpose(0, 2, 1)).astype(np.float32)
    if nonzero_affine:
        g_all = np.concatenate([ln1_g, ln2_g, lnf_g[None]], 0)    # (2L+1, E)
        b_all = np.concatenate([ln1_b, ln2_b, lnf_b[None]], 0)
        base["lngr"] = np.broadcast_to(
            g_all[:, None, :], (2 * L + 1, P, E)).astype(np.float32).copy()
        base["lnbr"] = np.broadcast_to(
            b_all[:, None, :], (2 * L + 1, P, E)).astype(np.float32).copy()

    in_maps = []
    for core in range(NCORES):
        b = core % B
        vh = core // B
        m = dict(base)
        m["x0"] = _rearr_pt(x0[b])
        m["maskT"] = _rearr_pt(np.ascontiguousarray(maskT[b]))
        m["wheadT"] = whT_halves[vh]
        in_maps.append(m)

    return in_maps, dict(nonzero_bias=nonzero_bias,
                         nonzero_affine=nonzero_affine), batch_mask


def kernel(**inputs):
    from concourse import bass_utils

    inputs = {k: np.asarray(v) for k, v in inputs.items()}
    in_maps, flags, batch_mask = _host_prep(**inputs)

    key = ("full", T, L, VSHARD, flags["nonzero_bias"], flags["nonzero_affine"])
    if key not in _CACHE:
        _CACHE[key] = _build_nc(dict(T=T, L=L, VS=VSHARD,
                                     nonzero_bias=flags["nonzero_bias"],
                                     nonzero_affine=flags["nonzero_affine"]))
    nc = _CACHE[key]

    res = bass_utils.run_bass_kernel_spmd(
        nc, in_maps, core_ids=list(range(NCORES)), trace=False)
    kernel._last_nc = nc

    out = np.empty((B, T, V), np.float32)
    for core in range(NCORES):
        b = core % B
        vh = core // B
        lg = res.results[core]["logits"]           # [P, TO, VS]
        out[b, :, vh * VSHARD:(vh + 1) * VSHARD] = (
            lg.transpose(1, 0, 2).reshape(T, VSHARD))
    out[batch_mask] = np.nan
    return out
